# revision 2
# baseline (speedup 1.0000x reference)
"""Cross-attention block kernel for Trainium2, 8 NeuronCores — fp8 version.

Sharding: 8 cores = 4 batches x 2 head-groups (8 heads / 512 local dims each).
All matmuls run in fp8e4 DoubleRow mode (2 k-tiles per instruction, 0.5
cycles/row): projections pair D-chunks, scores pair the two 32-dim halves of
each head (partition layout: 4 heads x 32 dims), attnV pairs seq chunks,
out-proj pairs dim chunks. Weights are host-quantized to fp8 at 16x scale;
descale factors fold into the PSUM->SBUF copies. exp runs on ACT (fp8 out,
bias=+2 rescale) with a fraction on DVE via a Schraudolph bit-trick into
fp8 bytes. LN normalize runs on GpSimd. bk is dropped (softmax-invariant);
bv folds into bo on the host; partials are 256x and divided on the host.
"""

import sys
import numpy as np

for _p in ("/opt/trn_rl_repo",):
    if _p not in sys.path:
        sys.path.insert(0, _p)

import concourse.bass as bass  # noqa: E402
import concourse.bacc as bacc  # noqa: E402
import concourse.tile as tile  # noqa: E402
from concourse import mybir  # noqa: E402
from concourse import bass_utils  # noqa: E402
from concourse.masks import make_identity  # noqa: E402

F32 = mybir.dt.float32
F32R = mybir.dt.float32r
FP8 = mybir.dt.float8e4
U8 = mybir.dt.uint8
NP8 = mybir.dt.np(FP8)
DRM = mybir.MatmulPerfMode.DoubleRow
P = 128
EPS = 1e-5
WS = 16.0                      # host weight pre-scale
SQ8 = np.sqrt(8.0)
EXP_BIAS = 2.0                 # exp(s + 2) rescale, cancelled by Z
SCH_A = 8.0 / np.log(2.0)
SCH_B = 8.0 * (7.0 + EXP_BIAS / np.log(2.0)) + 0.5 - 0.344
EXP_DVE_MOD = 4                # every 4th exp tile goes to DVE


def r(ap):
    return ap.bitcast(F32R)


def build_body(ctx, tc, outs, ins, cfg):
    nc = tc.nc
    S, D, LH, Hd = cfg["S"], cfg["D"], cfg["LH"], cfg["Hd"]
    LD = LH * Hd
    nST = S // P
    nDC = D // P
    nMT = LD // P                 # 4 slices of 128 output dims
    QC = 512
    nQC = S // QC
    NC_ = 512
    nNC = D // NC_
    nSQ = QC // P

    xq, xkv = ins["xq"], ins["xkv"]
    wq8, wk8, wv8, wo8 = ins["wq8"], ins["wk8"], ins["wv8"], ins["wo8"]
    bq4 = ins["bq4"]              # (P, nMT) f32, permuted+scaled
    out = outs["out_p"]

    singles = ctx.enter_context(tc.tile_pool(name="singles", bufs=1))
    xpool = ctx.enter_context(tc.tile_pool(name="xpool", bufs=3))
    lnp = ctx.enter_context(tc.tile_pool(name="lnp", bufs=2))
    xnp = ctx.enter_context(tc.tile_pool(name="xnp", bufs=2))
    bigA = ctx.enter_context(tc.tile_pool(name="bigA", bufs=1))
    bigB = ctx.enter_context(tc.tile_pool(name="bigB", bufs=1))
    ktp = ctx.enter_context(tc.tile_pool(name="ktp", bufs=1))
    vnp = ctx.enter_context(tc.tile_pool(name="vnp", bufs=1))
    wpool = ctx.enter_context(tc.tile_pool(name="wpool", bufs=2))
    wopool = ctx.enter_context(tc.tile_pool(name="wopool", bufs=1))
    expp = ctx.enter_context(tc.tile_pool(name="expp", bufs=4))
    stp = ctx.enter_context(tc.tile_pool(name="stp", bufs=2))
    recp = ctx.enter_context(tc.tile_pool(name="recp", bufs=2))
    atp = ctx.enter_context(tc.tile_pool(name="atp", bufs=2))
    opp = ctx.enter_context(tc.tile_pool(name="opp", bufs=2))
    dram = ctx.enter_context(tc.tile_pool(name="dram", bufs=1, space="DRAM"))

    psA = ctx.enter_context(tc.tile_pool(name="psA", bufs=2, space="PSUM"))
    pap = ctx.enter_context(tc.tile_pool(name="pap", bufs=2, space="PSUM"))
    pjp = ctx.enter_context(tc.tile_pool(name="pjp", bufs=2, space="PSUM"))

    ident = singles.tile([P, P], F32)
    make_identity(nc, ident)
    ones_r = singles.tile([P, 64], F32)
    nc.vector.memset(ones_r, 1.0)
    ones_rr = singles.tile([P, 64], F32R)
    nc.vector.tensor_copy(out=ones_rr, in_=ones_r)
    eps_t = singles.tile([P, 1], F32)
    nc.vector.memset(eps_t, EPS)
    bias2 = singles.tile([P, 1], F32)
    nc.vector.memset(bias2, EXP_BIAS)
    bq_sb = singles.tile([P, 6], F32)
    nc.sync.dma_start(out=bq_sb, in_=bq4)

    attnH_qc = []
    for _qc in range(nQC):
        ah = dram.tile([nMT, 2, 64, QC], FP8, tag=f"ah{_qc}",
                       name=f"attnH_{_qc}")
        attnH_qc.append(ah)

    cqk = float(1.0 / (WS * SQ8))

    def layernorm_T(x_dram, xT):
        """LN rows of x (S,D) -> fp8 transposed xT [P, nDC, S]."""
        for st in range(nST):
            xt = xpool.tile([P, D], F32, tag="x")
            nc.sync.dma_start(out=xt, in_=x_dram[st * P:(st + 1) * P, :])
            stats = lnp.tile([P, 2, 6], F32, tag="stats")
            xg = xt.rearrange("p (n s) -> p n s", n=2)
            for g in range(2):
                nc.vector.bn_stats(out=stats[:, g, :], in_=xg[:, g, :])
            mv = lnp.tile([P, 2], F32, tag="mv")
            nc.vector.bn_aggr(out=mv, in_=stats)
            rstd = lnp.tile([P, 1], F32, tag="rstd")
            nc.scalar.activation(out=rstd, in_=mv[:, 1:2],
                                 func=mybir.ActivationFunctionType.Sqrt,
                                 bias=eps_t)
            nc.vector.reciprocal(out=rstd, in_=rstd)
            xn = xnp.tile([P, D], F32, tag="xn")
            nc.gpsimd.tensor_scalar(out=xn, in0=xt, scalar1=mv[:, 0:1],
                                    scalar2=rstd,
                                    op0=mybir.AluOpType.subtract,
                                    op1=mybir.AluOpType.mult)
            pt = psA.tile([P, D], F32, tag="ps")
            for dc in range(nDC):
                nc.tensor.transpose(pt[:, dc * P:(dc + 1) * P],
                                    xn[:, dc * P:(dc + 1) * P], ident)
            nc.vector.tensor_copy(out=xT[:, :, st * P:(st + 1) * P],
                                  in_=pt.rearrange("p (c n) -> p c n", c=nDC))

    HPT = (3, 3, 2)               # heads per (tt) tile: bases 0/32/64 only

    def project_qk(xT, w_sb, b_sb, outT4, with_bias):
        """outT4 [P, 3, 2, S] (tt, j, seq); partition = 32*hh + r."""
        col0 = 0
        for tt in range(3):
            npp = 32 * HPT[tt]
            for j in range(2):
                si = 2 * tt + j
                for q in range(0, S, QC):
                    pj = pjp.tile([P, QC], F32, tag="pj")
                    for g in range(nDC // 2):
                        nc.tensor.matmul(
                            pj[0:npp, :],
                            w_sb[:, 2 * g:2 * g + 2, col0:col0 + npp],
                            xT[:, 2 * g:2 * g + 2, q:q + QC],
                            start=(g == 0), stop=(g == nDC // 2 - 1),
                            perf_mode=DRM)
                    if with_bias:
                        nc.vector.tensor_scalar(
                            out=outT4[0:npp, tt, j, q:q + QC],
                            in0=pj[0:npp, :],
                            scalar1=cqk, scalar2=b_sb[0:npp, si:si + 1],
                            op0=mybir.AluOpType.mult,
                            op1=mybir.AluOpType.add)
                    else:
                        nc.vector.tensor_scalar(
                            out=outT4[0:npp, tt, j, q:q + QC],
                            in0=pj[0:npp, :],
                            scalar1=cqk, scalar2=None,
                            op0=mybir.AluOpType.mult,
                            op1=mybir.AluOpType.bypass)
                col0 += npp

    def project_V(kvT, w_sb, VN):
        nc.vector.memset(VN[:, :, :, 64:65], WS)
        nc.vector.memset(VN[:, :, :, 65:66], 0.0)
        for st in range(nST):
            pj = pjp.tile([P, LD], F32, tag="pj")
            for g in range(nDC // 2):
                nc.tensor.matmul(
                    pj,
                    kvT[:, 2 * g:2 * g + 2, st * P:(st + 1) * P],
                    w_sb[:, 2 * g:2 * g + 2, 0:LD],
                    start=(g == 0), stop=(g == nDC // 2 - 1),
                    perf_mode=DRM)
            nc.vector.tensor_copy(
                out=VN[:, st, :, 0:64],
                in_=pj.rearrange("p (h d) -> p h d", d=Hd))

    # ---- q side ----
    qnT = bigA.tile([P, nDC, S], FP8, tag="bigA")
    layernorm_T(xq, qnT)
    w_sb = wpool.tile([P, nDC, LD], FP8, tag="w")
    nc.sync.dma_start(out=w_sb, in_=wq8.rearrange("(c p) n -> p c n", p=P))
    QT4 = bigB.tile([P, 3, 2, S], FP8, tag="bigB")
    project_qk(qnT, w_sb, bq_sb, QT4, True)
    # ---- kv side ----
    kvT = bigA.tile([P, nDC, S], FP8, tag="bigA")
    layernorm_T(xkv, kvT)
    w_sb = wpool.tile([P, nDC, LD], FP8, tag="w")
    nc.sync.dma_start(out=w_sb, in_=wk8.rearrange("(c p) n -> p c n", p=P))
    KT4 = ktp.tile([P, 3, 2, S], FP8)
    project_qk(kvT, w_sb, None, KT4, False)
    w_sb = wpool.tile([P, nDC, LD], FP8, tag="w")
    nc.sync.dma_start(out=w_sb, in_=wv8.rearrange("(c p) n -> p c n", p=P))
    VN = vnp.tile([P, nST, LH, 66], FP8)
    project_V(kvT, w_sb, VN)

    # ---- attention ----
    exp_ctr = 0
    for qc in range(nQC):
        q0 = qc * QC
        for m in range(nMT):
            h0, h1 = 2 * m, 2 * m + 1
            pa0 = pap.tile([66, QC], F32, tag="pa")
            pa1 = pap.tile([66, QC], F32, tag="pa")
            for kcp in range(nST // 2):
                for h, pa in ((h0, pa0), (h1, pa1)):
                    tt, hp = h // 3, 32 * (h % 3)
                    ps = psA.tile([P, 2, QC], F32, tag="ps")
                    for jj in range(2):
                        kc = 2 * kcp + jj
                        nc.tensor.matmul(
                            ps[:, jj, :],
                            KT4[hp:hp + 32, tt, :, kc * P:(kc + 1) * P],
                            QT4[hp:hp + 32, tt, :, q0:q0 + QC],
                            start=True, stop=True, perf_mode=DRM)
                    ex = expp.tile([P, 2, QC], FP8, tag="ex")
                    if exp_ctr % EXP_DVE_MOD == EXP_DVE_MOD - 1:
                        nc.vector.tensor_scalar(
                            out=ex.bitcast(U8), in0=ps,
                            scalar1=float(SCH_A), scalar2=float(SCH_B),
                            op0=mybir.AluOpType.mult,
                            op1=mybir.AluOpType.add)
                    else:
                        nc.scalar.activation(
                            out=ex, in_=ps,
                            func=mybir.ActivationFunctionType.Exp,
                            bias=bias2)
                    exp_ctr += 1
                    nc.tensor.matmul(
                        pa, VN[:, 2 * kcp:2 * kcp + 2, h, :], ex,
                        start=(kcp == 0), stop=(kcp == nST // 2 - 1),
                        perf_mode=DRM)
            for h, pa in ((h0, pa0), (h1, pa1)):
                rec = recp.tile([65, QC], F32R, tag="rec")
                with nc.allow_low_precision(reason="softmax reciprocal"):
                    nc.vector.reciprocal(out=rec[64:65, :], in_=pa[64:65, :])
                pb = pjp.tile([64, QC], F32, tag="pj")
                nc.tensor.matmul(pb, ones_rr[64:65, :], rec[64:65, :],
                                 start=True, stop=True)
                s65f = stp.tile([64, QC], F32, tag="stf")
                nc.scalar.activation(out=s65f, in_=pa[0:64, :],
                                     func=mybir.ActivationFunctionType.Copy,
                                     bias=0.0, scale=float(WS))
                s65 = stp.tile([64, QC], FP8, tag="st")
                nc.vector.tensor_tensor(out=s65, in0=s65f, in1=pb,
                                        op=mybir.AluOpType.mult)
                nc.sync.dma_start(out=attnH_qc[qc][h // 2, h % 2, :, :],
                                  in_=s65)
        # ---- out projection for this q-chunk ----
        if qc == 0:
            wo_sb = wopool.tile([P, nMT, D], FP8, tag="wo")
            nc.sync.dma_start(out=wo_sb,
                              in_=wo8.rearrange("(c p) n -> p c n", p=P))
        for sq in range(qc * nSQ, (qc + 1) * nSQ):
            s_in_qc = (sq - qc * nSQ) * P
            at = atp.tile([P, nMT, P], FP8, tag="at")
            for h2 in range(2):
                nc.sync.dma_start(
                    out=at[h2 * 64:(h2 + 1) * 64, :, :],
                    in_=attnH_qc[qc][:, h2, :,
                                     s_in_qc:s_in_qc + P].transpose([1, 0, 2]))
            for nch in range(nNC):
                po = pjp.tile([P, NC_], F32, tag="pj")
                for j in range(nMT // 2):
                    nc.tensor.matmul(
                        po, at[:, 2 * j:2 * j + 2, :],
                        wo_sb[:, 2 * j:2 * j + 2, nch * NC_:(nch + 1) * NC_],
                        start=(j == 0), stop=(j == nMT // 2 - 1),
                        perf_mode=DRM)
                ot = opp.tile([P, NC_], F32, tag="ot")
                nc.scalar.copy(out=ot, in_=po)
                nc.sync.dma_start(
                    out=out[sq * P:(sq + 1) * P, nch * NC_:(nch + 1) * NC_],
                    in_=ot)


def build_program(cfg):
    from contextlib import ExitStack
    nc = bacc.Bacc("TRN2", target_bir_lowering=False, debug=False,
                   enable_asserts=False)
    S, D, LH, Hd = cfg["S"], cfg["D"], cfg["LH"], cfg["Hd"]
    LD = LH * Hd
    nMT = LD // P
    ins = {
        "xq": nc.dram_tensor("xq", [S, D], F32, kind="ExternalInput").ap(),
        "xkv": nc.dram_tensor("xkv", [S, D], F32, kind="ExternalInput").ap(),
        "wq8": nc.dram_tensor("wq8", [D, LD], FP8, kind="ExternalInput").ap(),
        "wk8": nc.dram_tensor("wk8", [D, LD], FP8, kind="ExternalInput").ap(),
        "wv8": nc.dram_tensor("wv8", [D, LD], FP8, kind="ExternalInput").ap(),
        "wo8": nc.dram_tensor("wo8", [LD, D], FP8, kind="ExternalInput").ap(),
        "bq4": nc.dram_tensor("bq4", [P, 6], F32, kind="ExternalInput").ap(),
    }
    outs = {
        "out_p": nc.dram_tensor("out_p", [S, D], F32,
                                kind="ExternalOutput").ap(),
    }
    with tile.TileContext(nc) as tc:
        with ExitStack() as ctx:
            build_body(ctx, tc, outs, ins, cfg)
    nc.compile()
    return nc


def make_in_maps(inputs, cfg, n_cores=8):
    S, D, LH, Hd = cfg["S"], cfg["D"], cfg["LH"], cfg["Hd"]
    LD = LH * Hd
    nMT = LD // P
    f32 = np.float32
    q = np.asarray(inputs["query_input"], f32)
    kv = np.asarray(inputs["kv_input"], f32)
    B = q.shape[0]

    def fold(w, b, lnw, lnb):
        w = np.asarray(w, f32)
        b = np.asarray(b, f32)
        w_eff = w * np.asarray(lnw, f32)[None, :]
        b_eff = b + w @ np.asarray(lnb, f32)
        return w_eff, b_eff

    wq_e, bq_e = fold(inputs["wq"], inputs["bq"], inputs["ln_q_w"],
                      inputs["ln_q_b"])
    wk_e, _ = fold(inputs["wk"], inputs["bk"], inputs["ln_kv_w"],
                   inputs["ln_kv_b"])
    wv_e, _ = fold(inputs["wv"], inputs["bv"], inputs["ln_kv_w"],
                   inputs["ln_kv_b"])
    wo = np.asarray(inputs["wo"], f32)

    # column permutation for the (tt, j) 3+3+2 head-tile layout
    HPT = (3, 3, 2)
    order = []
    for tt in range(3):
        for j in range(2):
            for hh in range(HPT[tt]):
                for rr in range(32):
                    order.append(64 * (3 * tt + hh) + 32 * j + rr)
    perm = np.array(order, np.int64)

    groups_per_batch = n_cores // B
    in_maps = []
    for c in range(n_cores):
        b = c // groups_per_batch
        hg = c % groups_per_batch
        sl = slice(hg * LD, (hg + 1) * LD)
        wq_sl = (WS * wq_e[sl, :]).T[:, perm]          # (D, LD) permuted
        wk_sl = (WS * wk_e[sl, :]).T[:, perm]
        wv_sl = (WS * wv_e[sl, :]).T                   # natural
        wo_sl = (WS * wo[:, sl].T)                     # (LD, D)
        bq_p = (bq_e[sl] / SQ8)[perm]                  # permuted order
        bq_sl = np.zeros((P, 6), f32)
        col0 = 0
        for tt in range(3):
            npp = 32 * HPT[tt]
            for j in range(2):
                bq_sl[0:npp, 2 * tt + j] = bq_p[col0:col0 + npp]
                col0 += npp
        in_maps.append({
            "xq": np.ascontiguousarray(q[b]),
            "xkv": np.ascontiguousarray(kv[b]),
            "wq8": np.ascontiguousarray(wq_sl).astype(NP8),
            "wk8": np.ascontiguousarray(wk_sl).astype(NP8),
            "wv8": np.ascontiguousarray(wv_sl).astype(NP8),
            "wo8": np.ascontiguousarray(wo_sl).astype(NP8),
            "bq4": np.ascontiguousarray(bq_sl),
        })
    return in_maps


CFG_FULL = {"S": 2048, "D": 1024, "LH": 8, "Hd": 64}
_CACHE = {}
TRACE = False
LAST_RESULTS = None


def kernel(**inputs):
    cfg = CFG_FULL
    if "nc" not in _CACHE:
        _CACHE["nc"] = build_program(cfg)
    nc = _CACHE["nc"]
    in_maps = make_in_maps(inputs, cfg, n_cores=8)
    res = bass_utils.run_bass_kernel_spmd(
        nc, in_maps, core_ids=list(range(8)), trace=TRACE)
    global LAST_RESULTS
    LAST_RESULTS = res
    B = np.asarray(inputs["query_input"]).shape[0]
    gpb = 8 // B
    out = np.empty((B, cfg["S"], cfg["D"]), np.float32)
    wo = np.asarray(inputs["wo"], np.float32)
    bo_eff = (np.asarray(inputs["bo"], np.float32)
              + np.asarray(inputs["bv"], np.float32) @ wo.T)
    inv = np.float32(1.0 / (WS * WS))
    for b in range(B):
        acc = np.asarray(inputs["query_input"][b], np.float32) + bo_eff
        for g in range(gpb):
            acc = acc + res.results[b * gpb + g]["out_p"] * inv
        out[b] = acc
    return out
